# revision 4
# baseline (speedup 1.0000x reference)
"""Trainium2 Bass kernel for nn_Decoder (2-layer bidirectional-style LSTM
decoder + vocab projection), SPMD across 8 NeuronCores.

Strategy:
  - Host: embedding gather (sharding prep), weight repacking/transposition.
  - Device, per core (identical program, per-core data):
    * Gih0 = x_all @ wih_l0^T + b0, sharded over the 4096-wide gate axis
      (512 gates/core), AllGather -> every core has full Gih0.
    * Layer-0 recurrence (both directions), replicated over cores, batch on
      partitions, whh streamed as the moving operand.  h transposed via
      DMA-transpose; h^T stored to DRAM (x1^T) for the Gih1 precompute.
    * Gih1 from x1^T (gate-sharded + AllGather), layer-1 recurrence likewise,
      h1^T stored to DRAM (outs^T).
    * FC: logits chunk = outs @ fc_w[vslice]^T, vocab sharded 8 ways
      (4000 vocab rows/core).
  - Host: concat vocab slices, (s,b)->(b,s) reorder, add fc_b.
"""
import json
import os
import sys

sys.path.insert(0, "/opt/trn_rl_repo")

import ml_dtypes
import numpy as np

import concourse.bass as bass
import concourse.tile as tile
from concourse import mybir
from concourse.bass_utils import run_bass_kernel_spmd

BF16 = ml_dtypes.bfloat16
V, E, H, B, S = 32000, 512, 512, 64, 64
R = S * B              # 4096 rows, s-major: r = 64*s + b
NC = 8
VS = V // NC           # 4000 vocab rows per core
G2 = 8 * H             # 4096 = both cells' gates per layer
GB = G2 // NC          # 512 gates per core (AG mode)
USE_AG = os.environ.get("BASS_NO_AG", "") == ""
NB = 1 if USE_AG else 8          # gate blocks computed locally
GW = 512 * NB                    # local gih width

F32 = mybir.dt.float32
BF = mybir.dt.bfloat16


# --------------------------------------------------------------------------
# walrus workaround: this build allows at most 2 sem waits per instruction.
def _split_excess_waits(bir_json):
    j = json.loads(bir_json)
    n = 0
    for fn in j.get("functions", []):
        for blk in fn.get("blocks", []):
            out = []
            for inst in blk.get("instructions", []):
                si = inst.get("sync_info")
                ow = (si or {}).get("on_wait") or []
                keep = 2 if inst.get("opcode") == "EventSemaphore" else 1
                if len(ow) > keep:
                    extra, rest = ow[:-keep], ow[-keep:]
                    for i in range(0, len(extra), 2):
                        n += 1
                        out.append({
                            "debug": inst.get("debug", 0),
                            "engine": inst["engine"],
                            "ins": [], "outs": [],
                            "name": f"WSPLIT-{n}",
                            "opcode": "EventSemaphore",
                            "sync_info": {"on_update": [],
                                          "on_wait": extra[i:i + 2]},
                        })
                    si["on_wait"] = rest
                out.append(inst)
            blk["instructions"] = out
    return json.dumps(j).encode()


def _install_shim():
    import concourse.bass2jax as b2j
    import concourse.bass_utils as bu
    if getattr(bu, "_wsplit_installed", False):
        return
    orig = bu.compile_bir_kernel

    def patched(bir_json, tmpdir, neff_name="file.neff"):
        return orig(_split_excess_waits(bir_json), tmpdir, neff_name)

    bu.compile_bir_kernel = patched
    bu._wsplit_installed = True
    b2j.compile_bir_kernel = patched


# --------------------------------------------------------------------------
def build_nc():
    nc = bass.Bass()

    xfullT = nc.dram_tensor("xfullT", [128, 5, R], BF, kind="ExternalInput")
    wih0T = nc.dram_tensor("wih0T", [128, 5, GW], BF, kind="ExternalInput")
    wih1T = nc.dram_tensor("wih1T", [128, 9, GW], BF, kind="ExternalInput")
    whh0T = nc.dram_tensor("whh0T", [128, 4, G2], BF, kind="ExternalInput")
    whh1T = nc.dram_tensor("whh1T", [128, 4, G2], BF, kind="ExternalInput")
    fcwT = nc.dram_tensor("fcwT", [128, 8, VS], BF, kind="ExternalInput")
    hT0 = nc.dram_tensor("hT0", [128, 16, 64], BF, kind="ExternalInput")
    c0_in = nc.dram_tensor("c0", [64, 4, H], F32, kind="ExternalInput")
    eye_in = nc.dram_tensor("eye64", [64, 64], BF, kind="ExternalInput")
    ones_in = nc.dram_tensor("ones1", [128, R], BF, kind="ExternalInput")
    out = nc.dram_tensor("out", [R, VS], F32, kind="ExternalOutput")

    # h^T accumulators in DRAM: x1T[kc 0..7]=h0^T, chunk 8 = ones row (bias)
    x1T_d = nc.dram_tensor("x1T_d", [128, 9, R], BF)
    outsT_d = nc.dram_tensor("outsT_d", [128, 8, R], BF)
    gih_loc = [nc.dram_tensor(f"gih{l}_loc", [R, GW], BF) for l in (0, 1)]
    if USE_AG:
        gih_all = [
            nc.dram_tensor(f"gih{l}_all", [NC * R, GW], BF, addr_space="Shared")
            for l in (0, 1)
        ]

    with tile.TileContext(nc) as tc:
        with tc.tile_pool(name="persist", bufs=1) as persist:
            eye = persist.tile([64, 64], BF)
            nc.sync.dma_start(eye[:], eye_in[:])
            hTi = persist.tile([128, 16, 64], BF)
            nc.sync.dma_start(hTi[:], hT0[:])
            c_st = persist.tile([64, 4, H], F32)
            nc.sync.dma_start(c_st[:], c0_in[:])
            # ones row for x1T bias chunk (sent via DMA from input)
            nc.sync.dma_start(x1T_d[:, 8, :], ones_in[:])

            # ---- phase G0: Gih0 (gate-sharded), from host-gathered x ----
            with (
                tc.tile_pool(name="g0w", bufs=1) as g0w,
                tc.tile_pool(name="g0s", bufs=3) as g0s,
                tc.tile_pool(name="g0p", bufs=3, space="PSUM") as g0p,
            ):
                xT = g0w.tile([128, 5, R], BF)
                nc.sync.dma_start(xT[:], xfullT[:])
                w0 = g0w.tile([128, 5, GW], BF)
                nc.sync.dma_start(w0[:], wih0T[:])
                for m in range(32):
                    for nb in range(NB):
                        ps = g0p.tile([128, 512], F32)
                        for kc in range(5):
                            nc.tensor.matmul(
                                ps[:],
                                xT[:, kc, 128 * m:128 * (m + 1)],
                                w0[:, kc, 512 * nb:512 * (nb + 1)],
                                start=(kc == 0), stop=(kc == 4),
                            )
                        sb = g0s.tile([128, 512], BF)
                        nc.vector.tensor_copy(sb[:], ps[:])
                        nc.sync.dma_start(
                            gih_loc[0][128 * m:128 * (m + 1),
                                       512 * nb:512 * (nb + 1)], sb[:])
            if USE_AG:
                nc.gpsimd.collective_compute(
                    "AllGather", mybir.AluOpType.bypass,
                    ins=[gih_loc[0][:]], outs=[gih_all[0][:]],
                    replica_groups=[list(range(NC))],
                )

            # ---- recurrence (shared for both layers) ----
            def recurrence(layer, whh_sb, dstT):
                gl = layer
                with (
                    tc.tile_pool(name=f"rec{layer}_g", bufs=2) as gp_,
                    tc.tile_pool(name=f"rec{layer}_e", bufs=2) as ep,
                    tc.tile_pool(name=f"rec{layer}_h", bufs=2) as hp,
                    tc.tile_pool(name=f"rec{layer}_p", bufs=2,
                                 space="PSUM") as pp,
                ):
                    prev_hT = [None, None]
                    for s in range(S):
                        gihs = gp_.tile([64, G2], BF, tag="gih")
                        if USE_AG:
                            for rb in range(NC):
                                nc.sync.dma_start(
                                    gihs[0:64, 512 * rb:512 * (rb + 1)],
                                    gih_all[gl][R * rb + 64 * s:
                                                R * rb + 64 * (s + 1), :])
                        else:
                            nc.sync.dma_start(
                                gihs[0:64, :],
                                gih_loc[gl][64 * s:64 * (s + 1), :])
                        for c in range(2):
                            cell = 2 * layer + c
                            ps = pp.tile([64, 2048], F32)
                            for n in range(4):
                                nc.tensor.matmul(
                                    ps[:, 512 * n:512 * (n + 1)],
                                    eye[:],
                                    gihs[0:64, 2048 * c + 512 * n:
                                         2048 * c + 512 * (n + 1)],
                                    start=True, stop=False,
                                )
                            for k in range(4):
                                if s == 0:
                                    lhsT = hTi[:, 8 * layer + 4 * c + k, :]
                                else:
                                    lhsT = prev_hT[c][:, k, :]
                                for n in range(4):
                                    nc.tensor.matmul(
                                        ps[:, 512 * n:512 * (n + 1)],
                                        lhsT,
                                        whh_sb[:, k, 2048 * c + 512 * n:
                                               2048 * c + 512 * (n + 1)],
                                        start=False, stop=(k == 3),
                                    )
                            # elementwise: i f g o, each 512 wide
                            sif = ep.tile([64, 1024], F32, tag="sif")
                            nc.scalar.activation(
                                sif[:], ps[:, 0:1024],
                                mybir.ActivationFunctionType.Sigmoid)
                            tg = ep.tile([64, 512], F32, tag="tg")
                            nc.scalar.activation(
                                tg[:], ps[:, 1024:1536],
                                mybir.ActivationFunctionType.Tanh)
                            so = ep.tile([64, 512], F32, tag="so")
                            nc.scalar.activation(
                                so[:], ps[:, 1536:2048],
                                mybir.ActivationFunctionType.Sigmoid)
                            t1 = ep.tile([64, 512], F32, tag="t1")
                            nc.vector.tensor_mul(
                                t1[:], sif[:, 512:1024], c_st[:, cell, :])
                            t2 = ep.tile([64, 512], F32, tag="t2")
                            nc.vector.tensor_mul(t2[:], sif[:, 0:512], tg[:])
                            nc.vector.tensor_add(c_st[:, cell, :],
                                                 t1[:], t2[:])
                            tc2 = ep.tile([64, 512], F32, tag="tc2")
                            nc.scalar.activation(
                                tc2[:], c_st[:, cell, :],
                                mybir.ActivationFunctionType.Tanh)
                            hn = ep.tile([64, 512], BF, tag="hn")
                            nc.vector.tensor_mul(hn[:], so[:], tc2[:])
                            # h^T via DMA transpose (ring for next step)
                            hT = hp.tile([128, 4, 64], BF, tag=f"hT{c}")
                            for k in range(4):
                                nc.sync.dma_start(
                                    hT[:, k, :],
                                    hn[:, 128 * k:128 * (k + 1)],
                                    transpose=True,
                                )
                            prev_hT[c] = hT
                            # store h^T to DRAM (Gih1 / FC stationary)
                            nc.sync.dma_start(
                                dstT[:, 4 * c:4 * (c + 1),
                                     64 * s:64 * (s + 1)], hT[:])

            # ---- layer 0 ----
            with tc.tile_pool(name="whh0", bufs=1) as wp0:
                whh0 = wp0.tile([128, 4, G2], BF)
                nc.sync.dma_start(whh0[:], whh0T[:])
                recurrence(0, whh0, x1T_d)

            # ---- phase G1: Gih1 from x1T (streamed from DRAM) ----
            with (
                tc.tile_pool(name="g1w", bufs=1) as g1w,
                tc.tile_pool(name="g1x", bufs=3) as g1x,
                tc.tile_pool(name="g1s", bufs=3) as g1s,
                tc.tile_pool(name="g1p", bufs=3, space="PSUM") as g1p,
            ):
                w1 = g1w.tile([128, 9, GW], BF)
                nc.sync.dma_start(w1[:], wih1T[:])
                for m in range(32):
                    x1m = g1x.tile([128, 9, 128], BF, tag="x1m")
                    nc.sync.dma_start(
                        x1m[:], x1T_d[:, :, 128 * m:128 * (m + 1)])
                    for nb in range(NB):
                        ps = g1p.tile([128, 512], F32)
                        for kc in range(9):
                            nc.tensor.matmul(
                                ps[:],
                                x1m[:, kc, :],
                                w1[:, kc, 512 * nb:512 * (nb + 1)],
                                start=(kc == 0), stop=(kc == 8),
                            )
                        sb = g1s.tile([128, 512], BF)
                        nc.vector.tensor_copy(sb[:], ps[:])
                        nc.sync.dma_start(
                            gih_loc[1][128 * m:128 * (m + 1),
                                       512 * nb:512 * (nb + 1)], sb[:])
            if USE_AG:
                nc.gpsimd.collective_compute(
                    "AllGather", mybir.AluOpType.bypass,
                    ins=[gih_loc[1][:]], outs=[gih_all[1][:]],
                    replica_groups=[list(range(NC))],
                )

            # ---- layer 1 ----
            with tc.tile_pool(name="whh1", bufs=1) as wp1:
                whh1 = wp1.tile([128, 4, G2], BF)
                nc.sync.dma_start(whh1[:], whh1T[:])
                recurrence(1, whh1, outsT_d)

            # ---- FC ----
            with (
                tc.tile_pool(name="fcw", bufs=1) as fwp,
                tc.tile_pool(name="fcx", bufs=3) as fxp,
                tc.tile_pool(name="fco", bufs=2) as fop,
                tc.tile_pool(name="fcp", bufs=4, space="PSUM") as fpp,
            ):
                fcw = fwp.tile([128, 8, VS], BF)
                nc.sync.dma_start(fcw[:], fcwT[:])
                for m in range(32):
                    om = fxp.tile([128, 8, 128], BF, tag="om")
                    nc.sync.dma_start(
                        om[:], outsT_d[:, :, 128 * m:128 * (m + 1)])
                    ob = fop.tile([128, VS], F32, tag="ob")
                    for n in range(8):
                        ps = fpp.tile([128, 500], F32)
                        for k in range(8):
                            nc.tensor.matmul(
                                ps[:],
                                om[:, k, :],
                                fcw[:, k, 500 * n:500 * (n + 1)],
                                start=(k == 0), stop=(k == 7),
                            )
                        nc.vector.tensor_copy(ob[:, 500 * n:500 * (n + 1)],
                                              ps[:])
                    nc.sync.dma_start(out[128 * m:128 * (m + 1), :], ob[:])
    return nc


_NC_CACHE = None


def _pack_inputs(hidden_state, cell_state, Y, emb, w_ih_l0, w_hh_l0, b_ih_l0,
                 b_hh_l0, w_ih_l1, w_hh_l1, b_ih_l1, b_hh_l1, fc_w, fc_b):
    idx_seq = np.concatenate([Y[:, 1:2], Y[:, :-1]], axis=1)  # (B,S)
    idx_flat = idx_seq.T.reshape(-1).astype(np.int64)          # r = 64s + b
    x_all = np.asarray(emb, np.float32)[idx_flat]              # (R, E)

    def packT(w, kchunks, extra_row=None):
        gdim, kk = w.shape
        kc_data = kk // 128
        outp = np.zeros((128, kchunks, gdim), BF16)
        for kc in range(kc_data):
            outp[:, kc, :] = w[:, 128 * kc:128 * (kc + 1)].T.astype(BF16)
        if extra_row is not None:
            outp[0, kc_data, :] = extra_row.astype(BF16)
        return outp

    b0 = b_ih_l0 + b_hh_l0
    b1 = b_ih_l1 + b_hh_l1
    wih0_cat = np.vstack([w_ih_l0[0], w_ih_l0[1]]).astype(np.float32)
    wih1_cat = np.vstack([w_ih_l1[0], w_ih_l1[1]]).astype(np.float32)
    whh0_cat = np.vstack([w_hh_l0[0], w_hh_l0[1]]).astype(np.float32)
    whh1_cat = np.vstack([w_hh_l1[0], w_hh_l1[1]]).astype(np.float32)
    b0_cat = np.concatenate([b0[0], b0[1]]).astype(np.float32)
    b1_cat = np.concatenate([b1[0], b1[1]]).astype(np.float32)

    xfullT = np.zeros((128, 5, R), BF16)
    for kc in range(4):
        xfullT[:, kc, :] = x_all[:, 128 * kc:128 * (kc + 1)].T.astype(BF16)
    xfullT[0, 4, :] = BF16(1.0)

    whh0T = packT(whh0_cat, 4)
    whh1T = packT(whh1_cat, 4)

    hT0 = np.zeros((128, 16, 64), BF16)
    hs = np.asarray(hidden_state, np.float32)
    for cell in range(4):
        for k in range(4):
            hT0[:, 4 * cell + k, :] = \
                hs[cell][:, 128 * k:128 * (k + 1)].T.astype(BF16)
    c0 = np.ascontiguousarray(
        np.transpose(np.asarray(cell_state, np.float32), (1, 0, 2)))
    eye64 = np.eye(64, dtype=np.float32).astype(BF16)
    ones1 = np.zeros((128, R), BF16)
    ones1[0, :] = BF16(1.0)

    fc_w = np.asarray(fc_w, np.float32)
    ins = []
    for j in range(NC):
        if USE_AG:
            gsl = slice(GB * j, GB * (j + 1))
            wih0T_j = packT(wih0_cat[gsl], 5, b0_cat[gsl])
            wih1T_j = packT(wih1_cat[gsl], 9, b1_cat[gsl])
        else:
            wih0T_j = packT(wih0_cat, 5, b0_cat)
            wih1T_j = packT(wih1_cat, 9, b1_cat)
        fcs = fc_w[VS * j:VS * (j + 1)]           # (4000, 1024)
        fcwT_j = np.zeros((128, 8, VS), BF16)
        for k in range(8):
            fcwT_j[:, k, :] = fcs[:, 128 * k:128 * (k + 1)].T.astype(BF16)
        ins.append({
            "xfullT": xfullT, "wih0T": wih0T_j, "wih1T": wih1T_j,
            "whh0T": whh0T, "whh1T": whh1T, "fcwT": fcwT_j,
            "hT0": hT0, "c0": c0, "eye64": eye64, "ones1": ones1,
        })
    return ins


def kernel(**inputs):
    global _NC_CACHE
    _install_shim()
    if _NC_CACHE is None:
        _NC_CACHE = build_nc()
    nc = _NC_CACHE
    in_maps = _pack_inputs(**inputs)
    res = run_bass_kernel_spmd(nc, in_maps, list(range(NC)))
    parts = [np.asarray(res.results[j]["out"]) for j in range(NC)]
    logits = np.concatenate(parts, axis=1)          # (R, V), r = 64s+b
    logits = logits.reshape(S, B, V).transpose(1, 0, 2).reshape(B * S, V)
    logits = logits + np.asarray(inputs["fc_b"], np.float32)[None, :]
    return logits.astype(np.float32)
